# revision 14
# baseline (speedup 1.0000x reference)
"""GCN layer (nn_GCNLayer) on 8 TRN2 NeuronCores via Bass/Tile.

Reference math (f32):
    A_hat  = A + I
    D      = A_hat.sum(axis=1)                  # = rowsum(A) + 1
    d      = 1/sqrt(D + 1e-10)
    out    = relu((d[:,None] * A_hat * d[None,:]) @ (X @ W))

Rewritten to avoid materializing A_norm:
    Ys     = d[:,None] * (X @ W)                                 # [N, C]
    out[r] = relu(d[r] * (A[r,:] @ Ys + Ys[r]))                  # +Ys[r] is the +I diag

Sharding (8 cores): rows of A ([N/8, N]) and X ([N/8, F]); W replicated.
Per core:
  phase 0: XW_loc = X_shard @ W (bf16 matmul, f32 accum)
  phase 1: stream A_shard f32 from HBM once; per [128, CH] chunk:
           - ScalarE cast f32->bf16 with free-dim accumulation => rowsums
           - PE transpose 128x128 bf16 tiles -> PSUM; drain to SBUF-resident A^T
  boundary: d_loc = 1/sqrt(rowsum+1+1e-10); Ys_loc = d_loc*XW_loc;
           AllGather(Ys_loc bf16) -> Ys [N, C]; DMA back to SBUF
  phase 2: per 128-row stripe: accumulate 64 matmuls lhsT=A^T tile, rhs=Ys tile
           into PSUM; += Ys_loc[stripe]; relu(d*psum) -> out
"""

import os
import sys

import numpy as np

sys.path.insert(0, "/opt/trn_rl_repo")

from contextlib import ExitStack

from concourse import bacc, bass, mybir, tile
from concourse.bass_utils import run_bass_kernel_spmd
from concourse.masks import make_identity

F32 = mybir.dt.float32
BF16 = mybir.dt.bfloat16
AF = mybir.ActivationFunctionType


def _ensure_axon_ntff_hook():
    """run_bass_kernel_spmd(trace=True) under axon imports
    antenv.axon_hooks, which the container's antenv stub lacks. Provide it
    via sys.modules, driving NTFF capture through libaxon_pjrt.so ctypes."""
    try:
        import antenv.axon_hooks  # noqa: F401

        return
    except ImportError:
        pass
    import contextlib
    import ctypes
    import types

    mod = types.ModuleType("antenv.axon_hooks")
    state = {"hook": None}

    def _build(so_path):
        if not os.path.exists(so_path):
            return None
        lib = ctypes.CDLL(so_path)
        if not hasattr(lib, "axon_start_nrt_profile"):
            return None
        lib.axon_start_nrt_profile.argtypes = [
            ctypes.POINTER(ctypes.c_int64),
            ctypes.c_size_t,
        ]
        lib.axon_start_nrt_profile.restype = ctypes.c_int64
        lib.axon_stop_nrt_profile.argtypes = [ctypes.c_char_p]
        lib.axon_stop_nrt_profile.restype = ctypes.c_int64

        @contextlib.contextmanager
        def _hook(output_dir, device_ids):
            import jax

            jax.devices()
            if device_ids:
                ids = (ctypes.c_int64 * len(device_ids))(*device_ids)
                rc = lib.axon_start_nrt_profile(ids, len(device_ids))
            else:
                rc = lib.axon_start_nrt_profile(None, 0)
            if rc != 0:
                raise RuntimeError(f"axon_start_nrt_profile rc={rc}")
            try:
                yield
            finally:
                n = lib.axon_stop_nrt_profile(str(output_dir).encode())
                if n < 0:
                    raise RuntimeError(f"axon_stop_nrt_profile rc={n}")

        return _hook

    def set_axon_ntff_profile_hook(hook):
        state["hook"] = hook

    def get_axon_ntff_profile_hook():
        if state["hook"] is None:
            state["hook"] = _build(
                os.environ.get("AXON_PJRT_SO", "/opt/axon/libaxon_pjrt.so")
            )
        return state["hook"]

    mod.set_axon_ntff_profile_hook = set_axon_ntff_profile_hook
    mod.get_axon_ntff_profile_hook = get_axon_ntff_profile_hook
    sys.modules["antenv.axon_hooks"] = mod
    try:
        import antenv

        antenv.axon_hooks = mod
    except ImportError:
        pass

N, FDIM, CDIM = 8192, 512, 256
NCORES = 8


def build(n=N, fdim=FDIM, cdim=CDIM, ncores=NCORES, ch=1024):
    """Build the SPMD Bass program (identical on every core)."""
    R = n // ncores      # rows per core
    S = R // 128         # 128-row stripes per core
    KT = n // 128        # contraction tiles
    NCH = n // ch        # chunks per stripe
    FT = fdim // 128
    KPC = KT // ncores   # k-tiles owned per core (= S)
    assert KPC == S

    nc = bacc.Bacc(
        "TRN2", target_bir_lowering=False, debug=False, num_devices=ncores
    )
    A_d = nc.dram_tensor("A", [R, n], F32, kind="ExternalInput").ap()
    X_d = nc.dram_tensor("X", [R, fdim], F32, kind="ExternalInput").ap()
    W_d = nc.dram_tensor("W", [fdim, cdim], F32, kind="ExternalInput").ap()
    out_d = nc.dram_tensor("out", [R, cdim], F32, kind="ExternalOutput").ap()
    xw_in_d = nc.dram_tensor("xw_in", [R, cdim], BF16).ap()
    xw_out_d = nc.dram_tensor("xw_out", [n, cdim], BF16, addr_space="Shared").ap()
    d_in_d = nc.dram_tensor("d_in", [R], F32).ap()
    d_out_d = nc.dram_tensor("d_out", [n], F32, addr_space="Shared").ap()
    groups = [list(range(ncores))]

    with tile.TileContext(nc) as tc, ExitStack() as ctx:
        const_pool = ctx.enter_context(tc.tile_pool(name="const", bufs=1))
        ident = const_pool.tile([128, 128], BF16)
        make_identity(nc, ident[:])
        ident_f = const_pool.tile([128, 128], F32)
        make_identity(nc, ident_f[:])

        # Persistent big tensors.
        at_pool = ctx.enter_context(tc.tile_pool(name="atp", bufs=1))
        # A^T bf16, stripe-major: slice (s, kt) at free offset (s*KT + kt)*128
        AT = at_pool.tile([128, S * KT * 128], BF16)
        ys_pool = ctx.enter_context(tc.tile_pool(name="ysp", bufs=1))
        ys_sb = ys_pool.tile([128, KT * cdim], BF16)   # Ys, kt-major

        small_pool = ctx.enter_context(tc.tile_pool(name="small", bufs=1))
        xw_f32 = small_pool.tile([128, S * cdim], F32)   # XW_loc then Ys_loc (in place)
        xw_bf = small_pool.tile([128, S * cdim], BF16)
        Dacc = small_pool.tile([128, S * NCH], F32)
        Dsum = small_pool.tile([128, S], F32)
        d_loc = small_pool.tile([128, S], F32)
        d_kt = small_pool.tile([128, KT], F32)
        dT_sb = small_pool.tile([128, 128], F32)
        dg_sb = small_pool.tile([128, 128], F32)

        # ---- Phase 0: XW_loc = X_shard @ W (bf16) ----
        with tc.tile_pool(name="ph0", bufs=2) as ph0, \
             tc.tile_pool(name="ph0c", bufs=1) as ph0c, \
             tc.tile_pool(name="ph0ps", bufs=2, space="PSUM") as ph0ps:
            w_f32 = ph0c.tile([128, FT * cdim], F32)
            w_bf = ph0c.tile([128, FT * cdim], BF16)
            for f in range(FT):
                nc.sync.dma_start(
                    w_f32[:, f * cdim:(f + 1) * cdim],
                    W_d[f * 128:(f + 1) * 128, :],
                )
            nc.vector.tensor_copy(w_bf[:], w_f32[:])

            xT = ph0c.tile([128, S * FT * 128], BF16)  # X^T tiles, (s, f)
            for s in range(S):
                x_f32 = ph0.tile([128, fdim], F32)
                nc.sync.dma_start(x_f32[:], X_d[s * 128:(s + 1) * 128, :])
                x_bf = ph0.tile([128, fdim], BF16)
                nc.vector.tensor_copy(x_bf[:], x_f32[:])
                pxt = ph0ps.tile([128, fdim], BF16)
                for f in range(FT):
                    nc.tensor.transpose(
                        pxt[:, f * 128:(f + 1) * 128],
                        x_bf[:, f * 128:(f + 1) * 128],
                        ident[:],
                    )
                nc.scalar.copy(
                    xT[:, (s * FT) * 128:(s * FT + FT) * 128], pxt[:]
                )
            for s in range(S):
                pxw = ph0ps.tile([128, cdim], F32)
                for f in range(FT):
                    nc.tensor.matmul(
                        pxw[:],
                        lhsT=xT[:, (s * FT + f) * 128:(s * FT + f + 1) * 128],
                        rhs=w_bf[:, f * cdim:(f + 1) * cdim],
                        start=(f == 0),
                        stop=(f == FT - 1),
                    )
                nc.vector.tensor_copy(xw_f32[:, s * cdim:(s + 1) * cdim], pxw[:])

        # Early AllGather of (unscaled) XW in bf16, overlapped with phase 1.
        nc.vector.tensor_copy(xw_bf[:], xw_f32[:])
        nc.sync.dma_start(
            xw_in_d.rearrange("(s p) c -> p s c", p=128),
            xw_bf[:].rearrange("p (s c) -> p s c", s=S),
        )
        nc.gpsimd.collective_compute(
            "AllGather",
            mybir.AluOpType.bypass,
            replica_groups=groups,
            ins=[xw_in_d],
            outs=[xw_out_d],
        )
        KCH = KT // 8
        for b in range(8):
            nc.sync.dma_start(
                ys_sb[:, b * KCH * cdim:(b + 1) * KCH * cdim].rearrange(
                    "p (k c) -> p k c", k=KCH
                ),
                xw_out_d[b * KCH * 128:(b + 1) * KCH * 128, :].rearrange(
                    "(k p) c -> p k c", p=128
                ),
            )

        # ---- Phase 1: stream A, cast+rowsum, transpose into resident A^T ----
        TPC = ch // 128          # transposes per chunk
        GRP = 8                  # transposes per PSUM bank / drain
        with tc.tile_pool(name="ast", bufs=3) as ast, \
             tc.tile_pool(name="abf", bufs=3) as abf, \
             tc.tile_pool(name="dum", bufs=1, space="PSUM") as dum, \
             tc.tile_pool(name="tps", bufs=3, space="PSUM") as tps:
            junk = dum.tile([1, 128], F32)
            for s in range(S):
                for c in range(NCH):
                    a_ch = ast.tile([128, ch], F32)
                    nc.sync.dma_start(
                        a_ch[:], A_d[s * 128:(s + 1) * 128, c * ch:(c + 1) * ch]
                    )
                    a_bf = abf.tile([128, ch], BF16)
                    i = s * NCH + c
                    nc.scalar.activation(
                        a_bf[:], a_ch[:], AF.Copy, accum_out=Dacc[:, i:i + 1]
                    )
                    for g in range(TPC // GRP):
                        pt = tps.tile([128, GRP * 128], BF16)
                        for t in range(GRP):
                            nc.tensor.transpose(
                                pt[:, t * 128:(t + 1) * 128],
                                a_bf[:, (g * GRP + t) * 128:(g * GRP + t + 1) * 128],
                                ident[:],
                            )
                        kt0 = c * TPC + g * GRP
                        dst = AT[:, (s * KT + kt0) * 128:(s * KT + kt0 + GRP) * 128]
                        if (s * NCH + c) % 2 == 0:
                            nc.vector.tensor_copy(dst, pt[:])
                        else:
                            nc.scalar.copy(dst, pt[:])
                    # Real (non-transpose) matmul each chunk keeps the PE HAM
                    # activity monitor from re-throttling the clock to 1.2GHz
                    # (transpose-mode doesn't count as PE-busy). Accumulated +
                    # read once below so DCE keeps them.
                    nc.tensor.matmul(
                        junk[:], lhsT=ident[:, 0:1], rhs=ident[:],
                        start=(i == 0), stop=(i == S * NCH - 1),
                    )
            nc.vector.tensor_copy(dT_sb[:1, :], junk[:])

        # ---- Boundary: d locally, tiny d AllGather, scale Ys ----
        nc.vector.tensor_reduce(
            Dsum[:],
            Dacc[:].rearrange("p (s c) -> p s c", s=S),
            axis=mybir.AxisListType.X,
            op=mybir.AluOpType.add,
        )
        # Dsq = sqrt(D + 1 + 1e-10); d = 1/Dsq
        Dsq = small_pool.tile([128, S], F32)
        bias1 = small_pool.tile([128, 1], F32)
        nc.gpsimd.memset(bias1[:], 1.0 + 1e-10)
        nc.scalar.activation(Dsq[:], Dsum[:], AF.Sqrt, bias=bias1[:])
        nc.vector.reciprocal(d_loc[:], Dsq[:])
        # Ys_loc = d * XW_loc (in place, f32) for the +I diagonal term
        for s in range(S):
            nc.vector.tensor_scalar_mul(
                xw_f32[:, s * cdim:(s + 1) * cdim],
                xw_f32[:, s * cdim:(s + 1) * cdim],
                d_loc[:, s:s + 1],
            )
        # d_loc [128, S] -> transposed [S, 128] -> DRAM [R] in global row order
        with tc.tile_pool(name="bps", bufs=2, space="PSUM") as bps:
            pdT = bps.tile([S, 128], F32)
            nc.tensor.transpose(pdT[:], d_loc[:], ident_f[:])
            nc.vector.tensor_copy(dT_sb[:S, :], pdT[:])
            nc.sync.dma_start(d_in_d.rearrange("(s p) -> s p", p=128), dT_sb[:S, :])
            nc.gpsimd.collective_compute(
                "AllGather",
                mybir.AluOpType.bypass,
                replica_groups=groups,
                ins=[d_in_d],
                outs=[d_out_d],
            )
            # d_out [n] -> [KT, 128] sbuf -> transpose -> d_kt [128, KT]
            nc.sync.dma_start(
                dg_sb[:KT, :], d_out_d.rearrange("(m p) -> m p", p=128)
            )
            pdg = bps.tile([128, KT], F32)
            nc.tensor.transpose(pdg[:], dg_sb[:KT, :], ident_f[:KT, :KT])
            nc.vector.tensor_copy(d_kt[:], pdg[:])
        # Scale the gathered XW by d[k] per k-tile (in place, bf16)
        for kt in range(KT):
            nc.vector.tensor_scalar_mul(
                ys_sb[:, kt * cdim:(kt + 1) * cdim],
                ys_sb[:, kt * cdim:(kt + 1) * cdim],
                d_kt[:, kt:kt + 1],
            )

        # ---- Phase 2: out[s] = relu(d * (A_shard @ Ys + Ys_loc[s])) ----
        with tc.tile_pool(name="ops", bufs=2, space="PSUM") as ops, \
             tc.tile_pool(name="outp", bufs=3) as outp:
            for s in range(S):
                po = ops.tile([128, cdim], F32)
                for kt in range(KT):
                    nc.tensor.matmul(
                        po[:],
                        lhsT=AT[:, (s * KT + kt) * 128:(s * KT + kt + 1) * 128],
                        rhs=ys_sb[:, kt * cdim:(kt + 1) * cdim],
                        start=(kt == 0),
                        stop=(kt == KT - 1),
                    )
                nc.vector.tensor_add(
                    po[:], po[:], xw_f32[:, s * cdim:(s + 1) * cdim]
                )
                ot = outp.tile([128, cdim], F32)
                nc.scalar.activation(ot[:], po[:], AF.Relu, scale=d_loc[:, s:s + 1])
                nc.sync.dma_start(out_d[s * 128:(s + 1) * 128, :], ot[:])

    nc.compile()
    return nc


def build_v3(n=N, fdim=FDIM, cdim=CDIM, ncores=NCORES, ch=1024):
    """Overlapped variant: chunked d AllGathers let the main matmuls run
    interleaved with the A streaming/transpose phase instead of after it.

    Phase-2 work is emitted in 'bursts' as (A^T stripes, d chunks) become
    available, accumulating partial sums in SBUF (PSUM zero-regions are
    bank-granular, so 8 concurrent open groups don't fit alongside the
    transpose banks)."""
    R = n // ncores
    S = R // 128
    KT = n // 128
    NCH = n // ch
    FT = fdim // 128
    assert KT // ncores == S
    DCH = min(4, S)          # d-exchange chunks
    SPC = S // DCH           # stripes per d-chunk
    assert S % DCH == 0

    nc = bacc.Bacc(
        "TRN2", target_bir_lowering=False, debug=False, num_devices=ncores
    )
    A_d = nc.dram_tensor("A", [R, n], F32, kind="ExternalInput").ap()
    X_d = nc.dram_tensor("X", [R, fdim], F32, kind="ExternalInput").ap()
    W_d = nc.dram_tensor("W", [fdim, cdim], F32, kind="ExternalInput").ap()
    out_d = nc.dram_tensor("out", [R, cdim], F32, kind="ExternalOutput").ap()
    xw_in_d = nc.dram_tensor("xw_in", [R, cdim], BF16).ap()
    xw_out_d = nc.dram_tensor("xw_out", [n, cdim], BF16, addr_space="Shared").ap()
    d_in_d = [
        nc.dram_tensor(f"d_in{c}", [SPC * 128], F32).ap() for c in range(DCH)
    ]
    d_out_d = [
        nc.dram_tensor(
            f"d_out{c}", [ncores * SPC * 128], F32, addr_space="Shared"
        ).ap()
        for c in range(DCH)
    ]
    groups = [list(range(ncores))]

    with tile.TileContext(nc) as tc, ExitStack() as ctx:
        const_pool = ctx.enter_context(tc.tile_pool(name="const", bufs=1))
        ident = const_pool.tile([128, 128], BF16)
        make_identity(nc, ident[:])
        ident_f = const_pool.tile([128, 128], F32)
        make_identity(nc, ident_f[:])

        at_pool = ctx.enter_context(tc.tile_pool(name="atp", bufs=1))
        AT = at_pool.tile([128, S * KT * 128], BF16)
        ys_pool = ctx.enter_context(tc.tile_pool(name="ysp", bufs=1))
        ys_sb = ys_pool.tile([128, KT * cdim], BF16)

        small_pool = ctx.enter_context(tc.tile_pool(name="small", bufs=1))
        xw_f32 = small_pool.tile([128, S * cdim], F32)
        xw_bf = small_pool.tile([128, S * cdim], BF16)
        acc_sb = small_pool.tile([128, S * cdim], F32)
        Dacc = small_pool.tile([128, S * NCH], F32)
        Dsum = small_pool.tile([128, S], F32)
        Dsq = small_pool.tile([128, S], F32)
        d_loc = small_pool.tile([128, S], F32)
        d_kt = small_pool.tile([128, DCH * ncores * SPC], F32)
        dT_sb = small_pool.tile([128, 128], F32)
        dg_sb = small_pool.tile([128, 128], F32)
        bias1 = small_pool.tile([128, 1], F32)
        nc.gpsimd.memset(bias1[:], 1.0 + 1e-10)

        # ---- Phase 0: XW_loc = X_shard @ W (bf16) ----
        with tc.tile_pool(name="ph0", bufs=2) as ph0, \
             tc.tile_pool(name="ph0c", bufs=1) as ph0c, \
             tc.tile_pool(name="ph0ps", bufs=2, space="PSUM") as ph0ps:
            w_f32 = ph0c.tile([128, FT * cdim], F32)
            w_bf = ph0c.tile([128, FT * cdim], BF16)
            for f in range(FT):
                nc.sync.dma_start(
                    w_f32[:, f * cdim:(f + 1) * cdim],
                    W_d[f * 128:(f + 1) * 128, :],
                )
            nc.vector.tensor_copy(w_bf[:], w_f32[:])
            xT = ph0c.tile([128, S * FT * 128], BF16)
            for s in range(S):
                x_f32 = ph0.tile([128, fdim], F32)
                nc.sync.dma_start(x_f32[:], X_d[s * 128:(s + 1) * 128, :])
                x_bf = ph0.tile([128, fdim], BF16)
                nc.vector.tensor_copy(x_bf[:], x_f32[:])
                pxt = ph0ps.tile([128, fdim], BF16)
                for f in range(FT):
                    nc.tensor.transpose(
                        pxt[:, f * 128:(f + 1) * 128],
                        x_bf[:, f * 128:(f + 1) * 128],
                        ident[:],
                    )
                nc.scalar.copy(xT[:, (s * FT) * 128:(s * FT + FT) * 128], pxt[:])
            for s in range(S):
                pxw = ph0ps.tile([128, cdim], F32)
                for f in range(FT):
                    nc.tensor.matmul(
                        pxw[:],
                        lhsT=xT[:, (s * FT + f) * 128:(s * FT + f + 1) * 128],
                        rhs=w_bf[:, f * cdim:(f + 1) * cdim],
                        start=(f == 0),
                        stop=(f == FT - 1),
                    )
                nc.vector.tensor_copy(xw_f32[:, s * cdim:(s + 1) * cdim], pxw[:])

        # Early AllGather of (unscaled) XW in bf16.
        nc.vector.tensor_copy(xw_bf[:], xw_f32[:])
        nc.sync.dma_start(
            xw_in_d.rearrange("(s p) c -> p s c", p=128),
            xw_bf[:].rearrange("p (s c) -> p s c", s=S),
        )
        nc.gpsimd.collective_compute(
            "AllGather",
            mybir.AluOpType.bypass,
            replica_groups=groups,
            ins=[xw_in_d],
            outs=[xw_out_d],
        )
        KCH = KT // 8
        for b in range(8):
            nc.sync.dma_start(
                ys_sb[:, b * KCH * cdim:(b + 1) * KCH * cdim].rearrange(
                    "p (k c) -> p k c", k=KCH
                ),
                xw_out_d[b * KCH * 128:(b + 1) * KCH * 128, :].rearrange(
                    "(k p) c -> p k c", p=128
                ),
            )

        # ---- Phase 1 + interleaved phase 2 bursts ----
        TPC = ch // 128
        GRP = 8
        ready_kts: list = []
        burst_n = [0] * S

        with tc.tile_pool(name="ast", bufs=3) as ast, \
             tc.tile_pool(name="abf", bufs=3) as abf, \
             tc.tile_pool(name="dum", bufs=1, space="PSUM") as dum, \
             tc.tile_pool(name="bps", bufs=1, space="PSUM") as bps, \
             tc.tile_pool(name="bur", bufs=2, space="PSUM") as bur, \
             tc.tile_pool(name="tps", bufs=2, space="PSUM") as tps, \
             tc.tile_pool(name="outp", bufs=3) as outp:
            junk = dum.tile([1, 128], F32)

            def emit_burst(r, kts):
                if not kts:
                    return
                pb = bur.tile([128, cdim], F32, tag="burst")
                for idx, kt in enumerate(kts):
                    nc.tensor.matmul(
                        pb[:],
                        lhsT=AT[:, (r * KT + kt) * 128:(r * KT + kt + 1) * 128],
                        rhs=ys_sb[:, kt * cdim:(kt + 1) * cdim],
                        start=(idx == 0),
                        stop=(idx == len(kts) - 1),
                    )
                dst = acc_sb[:, r * cdim:(r + 1) * cdim]
                if burst_n[r] == 0:
                    nc.vector.tensor_copy(dst, pb[:])
                else:
                    nc.vector.tensor_add(dst, dst, pb[:])
                burst_n[r] += 1

            for s in range(S):
                for c in range(NCH):
                    a_ch = ast.tile([128, ch], F32)
                    nc.sync.dma_start(
                        a_ch[:], A_d[s * 128:(s + 1) * 128, c * ch:(c + 1) * ch]
                    )
                    a_bf = abf.tile([128, ch], BF16)
                    i = s * NCH + c
                    nc.scalar.activation(
                        a_bf[:], a_ch[:], AF.Copy, accum_out=Dacc[:, i:i + 1]
                    )
                    for g in range(TPC // GRP):
                        pt = tps.tile([128, GRP * 128], BF16)
                        for t in range(GRP):
                            nc.tensor.transpose(
                                pt[:, t * 128:(t + 1) * 128],
                                a_bf[:, (g * GRP + t) * 128:(g * GRP + t + 1) * 128],
                                ident[:],
                            )
                        kt0 = c * TPC + g * GRP
                        dst = AT[:, (s * KT + kt0) * 128:(s * KT + kt0 + GRP) * 128]
                        if (s * NCH + c) % 2 == 0:
                            nc.vector.tensor_copy(dst, pt[:])
                        else:
                            nc.scalar.copy(dst, pt[:])
                    nc.tensor.matmul(
                        junk[:], lhsT=ident[:, 0:1], rhs=ident[:],
                        start=(i == 0), stop=(i == S * NCH - 1),
                    )

                # New stripe ready: burst it against previously-ready k-tiles.
                emit_burst(s, ready_kts)

                if (s + 1) % SPC == 0:
                    dc = s // SPC
                    s0 = dc * SPC
                    # Local d for stripes [s0, s0+SPC)
                    nc.vector.tensor_reduce(
                        Dsum[:, s0:s0 + SPC],
                        Dacc[:, s0 * NCH:(s0 + SPC) * NCH].rearrange(
                            "p (s c) -> p s c", s=SPC
                        ),
                        axis=mybir.AxisListType.X,
                        op=mybir.AluOpType.add,
                    )
                    nc.scalar.activation(
                        Dsq[:, s0:s0 + SPC], Dsum[:, s0:s0 + SPC],
                        AF.Sqrt, bias=bias1[:],
                    )
                    nc.vector.reciprocal(
                        d_loc[:, s0:s0 + SPC], Dsq[:, s0:s0 + SPC]
                    )
                    # Ys_loc slice (diagonal term), in place f32
                    for sl in range(s0, s0 + SPC):
                        nc.vector.tensor_scalar_mul(
                            xw_f32[:, sl * cdim:(sl + 1) * cdim],
                            xw_f32[:, sl * cdim:(sl + 1) * cdim],
                            d_loc[:, sl:sl + 1],
                        )
                    # Exchange this chunk of d
                    pdT = bps.tile([SPC, 128], F32, tag="pdT")
                    nc.tensor.transpose(pdT[:], d_loc[:, s0:s0 + SPC], ident_f[:])
                    nc.vector.tensor_copy(dT_sb[:SPC, :], pdT[:])
                    nc.sync.dma_start(
                        d_in_d[dc].rearrange("(s p) -> s p", p=128),
                        dT_sb[:SPC, :],
                    )
                    nc.gpsimd.collective_compute(
                        "AllGather",
                        mybir.AluOpType.bypass,
                        replica_groups=groups,
                        ins=[d_in_d[dc]],
                        outs=[d_out_d[dc]],
                    )
                    M = ncores * SPC
                    nc.sync.dma_start(
                        dg_sb[:M, :], d_out_d[dc].rearrange("(m p) -> m p", p=128)
                    )
                    pdg = bps.tile([128, M], F32, tag="pdg")
                    nc.tensor.transpose(pdg[:], dg_sb[:M, :], ident_f[:M, :M])
                    nc.vector.tensor_copy(d_kt[:, dc * M:(dc + 1) * M], pdg[:])
                    # Scale the gathered XW tiles this chunk covers
                    new_kts = []
                    for m in range(M):
                        kt = (m // SPC) * S + s0 + (m % SPC)
                        new_kts.append(kt)
                        nc.vector.tensor_scalar_mul(
                            ys_sb[:, kt * cdim:(kt + 1) * cdim],
                            ys_sb[:, kt * cdim:(kt + 1) * cdim],
                            d_kt[:, dc * M + m:dc * M + m + 1],
                        )
                    # Burst the new k-tiles against all streamed stripes
                    for r in range(s + 1):
                        emit_burst(r, new_kts)
                    ready_kts.extend(new_kts)

            nc.vector.tensor_copy(dT_sb[:1, :], junk[:])

            # ---- Final: out[r] = relu(d * (acc + Ys_loc[r])) ----
            for r in range(S):
                assert burst_n[r] > 0
                dst = acc_sb[:, r * cdim:(r + 1) * cdim]
                nc.vector.tensor_add(dst, dst, xw_f32[:, r * cdim:(r + 1) * cdim])
                ot = outp.tile([128, cdim], F32)
                nc.scalar.activation(ot[:], dst, AF.Relu, scale=d_loc[:, r:r + 1])
                nc.sync.dma_start(out_d[r * 128:(r + 1) * 128, :], ot[:])

    nc.compile()
    return nc


_NC_CACHE = {}

VARIANT = os.environ.get("GCN_KERNEL_VARIANT", "v3")


def _get_nc(key=(N, FDIM, CDIM, NCORES)):
    k = (VARIANT, *key)
    if k not in _NC_CACHE:
        builder = build_v3 if VARIANT == "v3" else build
        _NC_CACHE[k] = builder(*key)
    return _NC_CACHE[k]


def kernel(X, A, W, trace=False, **kw):
    X = np.ascontiguousarray(np.asarray(X, dtype=np.float32))
    A = np.ascontiguousarray(np.asarray(A, dtype=np.float32))
    W = np.ascontiguousarray(np.asarray(W, dtype=np.float32))
    n = A.shape[0]
    ncores = NCORES
    R = n // ncores
    if trace:
        _ensure_axon_ntff_hook()
    nc = _get_nc((n, X.shape[1], W.shape[1], ncores))
    in_maps = [
        {
            "A": A[i * R:(i + 1) * R],
            "X": X[i * R:(i + 1) * R],
            "W": W,
        }
        for i in range(ncores)
    ]
    res = run_bass_kernel_spmd(nc, in_maps, list(range(ncores)), trace=trace, **kw)
    out = np.concatenate([res.results[i]["out"] for i in range(ncores)], axis=0)
    if trace:
        return out, res
    return out
